# revision 19
# baseline (speedup 1.0000x reference)
"""Trainium2 Bass kernel for nn_Attention_48687749267843.

Windowed-attention block: B=8, C=384, 12 heads x 32 dim, N=1024 tokens,
relative-position bias from a (63*63, 12) table.

Sharding: pure data-parallel over batch -- core b handles batch element b.
No collectives.

v2 design (ACT-exp is the bottleneck engine; everything else hides under it):
  - All inputs pre-cast to fp16 on HOST (x, wq/wk/wv/wproj already folded
    with softmax scale/4) -> no SBUF bounce copies, all matmuls fp16.
  - Scores PSUM hold s/4; exp(4u) via ACT activation scale=4.0.
  - Attention tiled (qc=256 queries): per (quad of 4 heads, qc, kc of 128
    keys): 4 row-packed score MMs -> [128,1024] PSUM (2 banks), one exp
    -> ar fp16; bias applied as at = ar * expB on DVE in 4x mode with
    kc-PAIR fused [128,2048] tiles; AV with ones-column (M=33, 2-col-packed)
    accumulating over kc into one [128,512] PSUM bank per (quad,qc).
  - Normalize: DVE reciprocal of denominator rows (PSUM) -> fp16, GPSIMD
    partition_broadcast (no DRAM bounce), 4 per-head [32,256] DVE mults
    into attn_mid fp16.
  - Out-proj per qc (interleaved, one qc behind attention emission).
PSUM: score pool 2x[128,1024] (4 banks) + small pool 4x[128,512] (4 banks).
"""

import sys

for _p in ("/opt/trn_rl_repo",):
    if _p not in sys.path:
        sys.path.insert(0, _p)

import numpy as np

import concourse.bass as bass
import concourse.bacc as bacc
import concourse.tile as tile
from concourse import mybir
from concourse.bass_utils import run_bass_kernel_spmd

DIM = 384
NUM_HEADS = 12
HEAD_DIM = 32
MID = NUM_HEADS * HEAD_DIM  # 384
N = 1024  # 32*32 tokens
B = 8
NCORES = 8
SCALE = HEAD_DIM ** -0.5

FP32 = mybir.dt.float32
FP16 = mybir.dt.float16

KT = DIM // 128  # 3 contraction chunks for the 1x1-conv matmuls
KC = N // 128  # 8 key chunks
NQUAD = NUM_HEADS // 4  # 3 head quads
QC = 4  # four 256-query chunks
QW = N // QC  # 256
COLOF = {0: 0, 1: 512, 2: 256, 3: 768}  # head -> col offset inside a 1024 slab

_CACHE = {}


def _emit_program():
    nc = bacc.Bacc("TRN2", target_bir_lowering=False, debug=False)

    x_d = nc.declare_dram_parameter("x", [DIM, N], FP16, isOutput=False)
    wqT_d = nc.declare_dram_parameter("wqT", [DIM, MID], FP16, isOutput=False)
    wkT_d = nc.declare_dram_parameter("wkT", [DIM, MID], FP16, isOutput=False)
    wvT_d = nc.declare_dram_parameter("wvT", [DIM, MID], FP16, isOutput=False)
    wpT_d = nc.declare_dram_parameter("wpT", [MID, DIM], FP16, isOutput=False)
    # [quad][qc][kcp][key][kcj*1024 + hh*256 + q] -- each [128, 2048] tile is
    # a contiguous 512 KiB block (one clean DMA, 4KiB per partition line).
    expB_d = nc.declare_dram_parameter(
        "expBTr", [NQUAD, QC, KC // 2, 128, 2048], FP16, isOutput=False
    )
    out_d = nc.declare_dram_parameter("out", [DIM, N], FP32, isOutput=True)

    with tile.TileContext(nc) as tc:
        with (
            tc.tile_pool(name="persist", bufs=1) as persist,
            tc.tile_pool(name="ebt", bufs=10) as ebt_pool,
            tc.tile_pool(name="ar", bufs=3) as ar_pool,
            tc.tile_pool(name="at", bufs=3) as at_pool,
            tc.tile_pool(name="small", bufs=8) as small,
            tc.tile_pool(name="ob", bufs=3) as ob_pool,
            tc.tile_pool(name="ps_score", bufs=2, space="PSUM") as ps_score,
            tc.tile_pool(name="ps_small", bufs=4, space="PSUM") as ps_small,
        ):
            # ---- load x and weights (fp16, straight from DMA) ----
            x_sb = []
            for i in range(KT):
                t = persist.tile([128, N], FP16, name=f"x{i}", tag=f"x{i}")
                nc.sync.dma_start(out=t[:], in_=x_d[i * 128 : (i + 1) * 128, :])
                x_sb.append(t)

            def load_w(dram, name):
                tiles = []
                for i in range(KT):
                    t = persist.tile(
                        [128, MID], FP16, name=f"{name}{i}", tag=f"{name}{i}"
                    )
                    nc.sync.dma_start(
                        out=t[:], in_=dram[i * 128 : (i + 1) * 128, :]
                    )
                    tiles.append(t)
                return tiles

            wqT_sb = load_w(wqT_d, "wqT")
            wkT_sb = load_w(wkT_d, "wkT")
            wvT_sb = load_w(wvT_d, "wvT")
            wpT_sb = load_w(wpT_d, "wpT")

            # ---- q/k projections: out [MID, N] fp16 (q holds q*scale/4) ----
            # Split per head-pair ([64, N]) so heads 2,3 sit at partitions
            # 0:64 of their own tile: their score MMs then target PE row
            # groups 0/32 (same as heads 0,1) and serialize pairwise -- at
            # most 2 concurrent PSUM-bank writers, each on its own bank.
            q_sb = [
                [
                    persist.tile([64, N], FP16, name=f"q{i}_{s}", tag=f"q{i}_{s}")
                    for s in range(2)
                ]
                for i in range(KT)
            ]
            k_sb = [
                [
                    persist.tile([64, N], FP16, name=f"k{i}_{s}", tag=f"k{i}_{s}")
                    for s in range(2)
                ]
                for i in range(KT)
            ]

            def emit_qk_proj(mt):
                for (wt, dst) in ((wqT_sb, q_sb), (wkT_sb, k_sb)):
                    for half in range(2):
                        ps = ps_small.tile([128, 512], FP32, name="qk_ps", tag="ps_sm")
                        for kc in range(KT):
                            nc.tensor.matmul(
                                out=ps[:],
                                lhsT=wt[kc][:, mt * 128 : (mt + 1) * 128],
                                rhs=x_sb[kc][:, half * 512 : (half + 1) * 512],
                                start=(kc == 0),
                                stop=(kc == KT - 1),
                            )
                        for s in range(2):
                            nc.vector.tensor_copy(
                                out=dst[mt][s][:, half * 512 : (half + 1) * 512],
                                in_=ps[s * 64 : (s + 1) * 64, :],
                            )

            emit_qk_proj(0)

            # ---- vT = x^T @ wvT: [N, MID] fp16, interleaved with ones ----
            vT_sb = [
                persist.tile([128, NUM_HEADS * 33], FP16, name=f"vT{i}", tag=f"vT{i}")
                for i in range(KC)
            ]
            for kt in range(KC):
                ps = ps_small.tile([128, 512], FP32, name="v_ps", tag="ps_sm")
                for kc in range(KT):
                    nc.tensor.matmul(
                        out=ps[:, 0:MID],
                        lhsT=x_sb[kc][:, kt * 128 : (kt + 1) * 128],
                        rhs=wvT_sb[kc][:],
                        start=(kc == 0),
                        stop=(kc == KT - 1),
                    )
                dst3 = vT_sb[kt][:].rearrange("p (h c) -> p h c", h=NUM_HEADS)
                src3 = ps[:, 0:MID].rearrange("p (h c) -> p h c", h=NUM_HEADS)
                nc.vector.tensor_copy(out=dst3[:, :, 0:32], in_=src3)
                nc.vector.memset(dst3[:, :, 32:33], 1.0)

            # ---- attention ----
            attn_mid = [
                persist.tile([128, N], FP16, name=f"am{i}", tag=f"am{i}")
                for i in range(KT)
            ]

            def emit_out_proj(qc):
                # PSUM from the score pool: its slot is long free at the qc
                # boundary, so these MMs never block the in-order PE queue.
                q0 = qc * QW
                ps = ps_score.tile([128, 1024], FP32, name="op_ps", tag="st")
                OPC = (0, 256, 512)  # mt0/mt1 share bank0 sequentially
                for mt in range(KT):
                    c0 = OPC[mt]
                    for ch in range(KT):
                        nc.tensor.matmul(
                            out=ps[:, c0 : c0 + 256],
                            lhsT=wpT_sb[ch][:, mt * 128 : (mt + 1) * 128],
                            rhs=attn_mid[ch][:, q0 : q0 + QW],
                            start=(ch == 0),
                            stop=(ch == KT - 1),
                        )
                for mt in range(KT):
                    c0 = OPC[mt]
                    ob = ob_pool.tile([128, 256], FP32, name=f"ob{mt}", tag="ob")
                    nc.vector.tensor_copy(out=ob[:], in_=ps[:, c0 : c0 + 256])
                    nc.sync.dma_start(
                        out=out_d[mt * 128 : (mt + 1) * 128, q0 : q0 + QW],
                        in_=ob[:],
                    )

            pending = [[]]

            def attention_unit(qc, quad):
                q0 = qc * QW
                ebts = []
                for kcp in range(KC // 2):
                    ebt = ebt_pool.tile([128, 2048], FP16, name=f"ebt{kcp}", tag="ebt")
                    nc.sync.dma_start(out=ebt[:], in_=expB_d[quad, qc, kcp])
                    ebts.append(ebt)

                # one PSUM bank per head: single accumulation group per bank
                avs = [
                    ps_small.tile([128, 512], FP32, name=f"av{hh}", tag="ps_sm")
                    for hh in range(4)
                ]
                ats = [None] * KC
                ars = [None]

                def emit_av(kc):
                    at = ats[kc]
                    coff = (kc % 2) * 1024
                    for hh in range(4):
                        h = 4 * quad + hh
                        base = 64 * (hh % 2)
                        nc.tensor.matmul(
                            out=avs[hh][base : base + 33, 0:256],
                            lhsT=vT_sb[kc][:, h * 33 : h * 33 + 33],
                            rhs=at[:, coff + COLOF[hh] : coff + COLOF[hh] + 256],
                            start=(kc == 0),
                            stop=(kc == KC - 1),
                            tile_position=(0, base),
                        )

                for kc in range(KC):
                    kcp, kcj = kc // 2, kc % 2
                    st = ps_score.tile([128, 1024], FP32, name="st", tag="st")
                    # col layout: bank0 = [hh0 | hh2], bank1 = [hh1 | hh3];
                    # (hh0,hh1) run concurrently on distinct banks, then
                    # (hh2,hh3) reuse row groups 0/32 -> serialize after.
                    for hh in (0, 1, 2, 3):
                        s, r = hh // 2, (hh % 2) * 32
                        c0 = COLOF[hh]
                        nc.tensor.matmul(
                            out=st[:, c0 : c0 + 256],
                            lhsT=k_sb[quad][s][
                                r : r + 32, kc * 128 : (kc + 1) * 128
                            ],
                            rhs=q_sb[quad][s][r : r + 32, q0 : q0 + QW],
                            start=True,
                            stop=True,
                            tile_position=(r, 0),
                        )
                    # AV lags 3 kc so the pair-fused `at` is ready; 4-MM
                    # batches fit the PE wait queue without blocking scores.
                    # The previous unit's tail (AV 5/6/7 + normalize) is
                    # spread one piece per slot across kc 0..3.
                    if pending[0]:
                        pending[0].pop(0)()
                    if kc >= 3:
                        emit_av(kc - 3)
                    if kcj == 0:
                        ar = ar_pool.tile([128, 2048], FP16, name="ar", tag="ar")
                        ars[0] = ar
                    else:
                        ar = ars[0]
                    nc.scalar.activation(
                        out=ar[:, kcj * 1024 : (kcj + 1) * 1024],
                        in_=st[:],
                        func=mybir.ActivationFunctionType.Exp,
                        scale=4.0,
                    )
                    if kcj == 1:
                        at = at_pool.tile([128, 2048], FP16, name="at", tag="at")
                        nc.vector.tensor_tensor(
                            at[:], ar[:], ebts[kcp][:], mybir.AluOpType.mult
                        )
                        ats[2 * kcp] = at
                        ats[2 * kcp + 1] = at

                def norm():
                    # per-head pipelined normalize: each head's bank is
                    # released ~2us after the last AV instead of all four
                    # waiting for a batched chain.
                    for hh in range(4):
                        base = 64 * (hh % 2)
                        den = small.tile([1, 256], FP32, name=f"den{hh}", tag="den")
                        nc.vector.tensor_copy(
                            out=den[:], in_=avs[hh][base + 32 : base + 33, 0:256]
                        )
                        dsb = small.tile([1, 256], FP32, name=f"dsb{hh}", tag="dsb")
                        nc.vector.reciprocal_approx_fast(out=dsb[:], in_=den[:])
                        rb = small.tile([32, 256], FP32, name=f"rb{hh}", tag="rb")
                        nc.gpsimd.partition_broadcast(rb[:], dsb[0:1, :], channels=32)
                        nc.vector.tensor_tensor(
                            attn_mid[quad][hh * 32 : (hh + 1) * 32, q0 : q0 + QW],
                            avs[hh][base : base + 32, 0:256],
                            rb[:],
                            mybir.AluOpType.mult,
                        )

                pending[0] = [
                    lambda: emit_av(KC - 3),
                    lambda: emit_av(KC - 2),
                    lambda: emit_av(KC - 1),
                    norm,
                ]

            for qc in range(QC):
                for quad in range(NQUAD):
                    attention_unit(qc, quad)
                    # overlap the remaining projections with early attention
                    if qc == 0 and quad < 2:
                        emit_qk_proj(quad + 1)
                # out-proj one qc behind (deps long resolved -> no PE stall)
                if qc > 0:
                    emit_out_proj(qc - 1)
            for fn in pending[0]:
                fn()
            pending[0] = []
            emit_out_proj(QC - 1)
    nc.compile()
    return nc


def _prep_host(x, wq, bq, wkv, bkv, wproj, bproj, bias_table, rel_index):
    """Host-side input prep shared by all cores (weights / bias tables)."""
    wq = np.asarray(wq, np.float32) * np.float32(SCALE / 4.0)
    wkv = np.asarray(wkv, np.float32)
    wqT = np.ascontiguousarray(wq.T).astype(np.float16)
    wkT = np.ascontiguousarray(wkv[:MID].T).astype(np.float16)
    wvT = np.ascontiguousarray(wkv[MID:].T).astype(np.float16)
    wpT = np.ascontiguousarray(np.asarray(wproj, np.float32).T).astype(np.float16)
    # rel bias -> exp(bias) (exp-trick): expBT[h, j, i] = exp(B[i, j, h])
    bt = np.asarray(bias_table, np.float32)
    ri = np.asarray(rel_index, np.int64)
    Bfull = bt[ri.reshape(-1)].reshape(N, N, NUM_HEADS)  # i, j, h
    expBT = np.exp(Bfull.transpose(2, 1, 0)).astype(np.float16)  # h, j, i
    # -> [quad][qc][kcp][key][kcj*1024 + hh*256 + q], [128,2048] contiguous
    expBTr = np.ascontiguousarray(
        expBT.reshape(NQUAD, 4, KC // 2, 2, 128, QC, QW)[:, (0, 2, 1, 3)].transpose(
            0, 5, 2, 4, 3, 1, 6
        )
    ).reshape(NQUAD, QC, KC // 2, 128, 2048)
    return wqT, wkT, wvT, wpT, expBTr


def _install_ntff_hook():
    """The image's antenv lacks axon_hooks; reconstruct it so trace=True works."""
    import types, importlib.util

    try:
        from antenv.axon_hooks import get_axon_ntff_profile_hook  # noqa

        return
    except ImportError:
        pass
    import antenv

    mod = types.ModuleType("antenv.axon_hooks")
    _state = {"hook": None}
    mod.set_axon_ntff_profile_hook = lambda h: _state.__setitem__("hook", h)
    mod.get_axon_ntff_profile_hook = lambda: _state["hook"]
    sys.modules["antenv.axon_hooks"] = mod
    antenv.axon_hooks = mod

    spec = importlib.util.spec_from_file_location(
        "trn_boot", "/root/.axon_site/trn_agent_boot/trn_boot.py"
    )
    tb = importlib.util.module_from_spec(spec)
    spec.loader.exec_module(tb)
    mod.set_axon_ntff_profile_hook(
        tb._ntff_profile_via_ctypes("/opt/axon/libaxon_pjrt.so")
    )


def _run(inputs, trace=False):
    if trace:
        _install_ntff_hook()
    if "nc" not in _CACHE:
        _CACHE["nc"] = _emit_program()
    nc = _CACHE["nc"]

    x = np.asarray(inputs["x"], np.float32)
    wqT, wkT, wvT, wpT, expBTr = _prep_host(**inputs)

    in_maps = []
    for b in range(NCORES):
        in_maps.append(
            {
                "x": np.ascontiguousarray(x[b].reshape(DIM, N)).astype(np.float16),
                "wqT": wqT,
                "wkT": wkT,
                "wvT": wvT,
                "wpT": wpT,
                "expBTr": expBTr,
            }
        )
    res = run_bass_kernel_spmd(nc, in_maps, list(range(NCORES)), trace=trace)
    out = np.stack(
        [np.asarray(res.results[b]["out"]).reshape(DIM, 32, 32) for b in range(B)]
    )
    return out.astype(np.float32), res


def kernel(**inputs) -> np.ndarray:
    out, _ = _run(inputs, trace=False)
    return out


def kernel_traced(**inputs):
    """Returns (out, BassKernelResults) with profiling enabled."""
    return _run(inputs, trace=True)


# revision 20
# speedup vs baseline: 1.1685x; 1.1685x over previous
"""Trainium2 Bass kernel for nn_Attention_48687749267843.

Windowed-attention block: B=8, C=384, 12 heads x 32 dim, N=1024 tokens,
relative-position bias from a (63*63, 12) table.

Sharding: pure data-parallel over batch -- core b handles batch element b.
No collectives.

v2 design (ACT-exp is the bottleneck engine; everything else hides under it):
  - All inputs pre-cast to fp16 on HOST (x, wq/wk/wv/wproj already folded
    with softmax scale/4) -> no SBUF bounce copies, all matmuls fp16.
  - Scores PSUM hold s/4; exp(4u) via ACT activation scale=4.0.
  - Attention tiled (qc=256 queries): per (quad of 4 heads, qc, kc of 128
    keys): 4 row-packed score MMs -> [128,1024] PSUM (2 banks), one exp
    -> ar fp16; bias applied as at = ar * expB on DVE in 4x mode with
    kc-PAIR fused [128,2048] tiles; AV with ones-column (M=33, 2-col-packed)
    accumulating over kc into one [128,512] PSUM bank per (quad,qc).
  - Normalize: DVE reciprocal of denominator rows (PSUM) -> fp16, GPSIMD
    partition_broadcast (no DRAM bounce), 4 per-head [32,256] DVE mults
    into attn_mid fp16.
  - Out-proj per qc (interleaved, one qc behind attention emission).
PSUM: score pool 2x[128,1024] (4 banks) + small pool 4x[128,512] (4 banks).
"""

import sys

for _p in ("/opt/trn_rl_repo",):
    if _p not in sys.path:
        sys.path.insert(0, _p)

import numpy as np

import concourse.bass as bass
import concourse.bacc as bacc
import concourse.tile as tile
from concourse import mybir
from concourse.bass_utils import run_bass_kernel_spmd

DIM = 384
NUM_HEADS = 12
HEAD_DIM = 32
MID = NUM_HEADS * HEAD_DIM  # 384
N = 1024  # 32*32 tokens
B = 8
NCORES = 8
SCALE = HEAD_DIM ** -0.5

FP32 = mybir.dt.float32
FP16 = mybir.dt.float16

KT = DIM // 128  # 3 contraction chunks for the 1x1-conv matmuls
KC = N // 128  # 8 key chunks
NQUAD = NUM_HEADS // 4  # 3 head quads
QC = 4  # four 256-query chunks
QW = N // QC  # 256
COLOF = {0: 0, 1: 512, 2: 256, 3: 768}  # head -> col offset inside a 1024 slab

_CACHE = {}


def _emit_program():
    nc = bacc.Bacc("TRN2", target_bir_lowering=False, debug=False)

    x_d = nc.declare_dram_parameter("x", [DIM, N], FP16, isOutput=False)
    wqT_d = nc.declare_dram_parameter("wqT", [DIM, MID], FP16, isOutput=False)
    wkT_d = nc.declare_dram_parameter("wkT", [DIM, MID], FP16, isOutput=False)
    wvT_d = nc.declare_dram_parameter("wvT", [DIM, MID], FP16, isOutput=False)
    wpT_d = nc.declare_dram_parameter("wpT", [MID, DIM], FP16, isOutput=False)
    # [quad][qc][kcp][key][kcj*1024 + hh*256 + q] -- each [128, 2048] tile is
    # a contiguous 512 KiB block (one clean DMA, 4KiB per partition line).
    expB_d = nc.declare_dram_parameter(
        "expBTr", [NQUAD, QC, KC // 2, 128, 2048], FP16, isOutput=False
    )
    out_d = nc.declare_dram_parameter("out", [DIM, N], FP32, isOutput=True)

    with tile.TileContext(nc) as tc:
        with (
            tc.tile_pool(name="persist", bufs=1) as persist,
            tc.tile_pool(name="ebt", bufs=10) as ebt_pool,
            tc.tile_pool(name="ar", bufs=3) as ar_pool,
            tc.tile_pool(name="at", bufs=3) as at_pool,
            tc.tile_pool(name="small", bufs=8) as small,
            tc.tile_pool(name="ob", bufs=3) as ob_pool,
            tc.tile_pool(name="ps_score", bufs=2, space="PSUM") as ps_score,
            tc.tile_pool(name="ps_small", bufs=4, space="PSUM") as ps_small,
        ):
            # ---- load x and weights (fp16, straight from DMA) ----
            x_sb = []
            for i in range(KT):
                t = persist.tile([128, N], FP16, name=f"x{i}", tag=f"x{i}")
                nc.sync.dma_start(out=t[:], in_=x_d[i * 128 : (i + 1) * 128, :])
                x_sb.append(t)

            def load_w(dram, name):
                tiles = []
                for i in range(KT):
                    t = persist.tile(
                        [128, MID], FP16, name=f"{name}{i}", tag=f"{name}{i}"
                    )
                    nc.sync.dma_start(
                        out=t[:], in_=dram[i * 128 : (i + 1) * 128, :]
                    )
                    tiles.append(t)
                return tiles

            wqT_sb = load_w(wqT_d, "wqT")
            wkT_sb = load_w(wkT_d, "wkT")
            wvT_sb = load_w(wvT_d, "wvT")
            wpT_sb = load_w(wpT_d, "wpT")

            # ---- q/k projections: out [MID, N] fp16 (q holds q*scale/4) ----
            # Split per head-pair ([64, N]) so heads 2,3 sit at partitions
            # 0:64 of their own tile: their score MMs then target PE row
            # groups 0/32 (same as heads 0,1) and serialize pairwise -- at
            # most 2 concurrent PSUM-bank writers, each on its own bank.
            q_sb = [
                [
                    persist.tile([64, N], FP16, name=f"q{i}_{s}", tag=f"q{i}_{s}")
                    for s in range(2)
                ]
                for i in range(KT)
            ]
            k_sb = [
                [
                    persist.tile([64, N], FP16, name=f"k{i}_{s}", tag=f"k{i}_{s}")
                    for s in range(2)
                ]
                for i in range(KT)
            ]

            def emit_qk_proj(mt):
                for (wt, dst) in ((wqT_sb, q_sb), (wkT_sb, k_sb)):
                    for half in range(2):
                        ps = ps_small.tile([128, 512], FP32, name="qk_ps", tag="ps_sm")
                        for kc in range(KT):
                            nc.tensor.matmul(
                                out=ps[:],
                                lhsT=wt[kc][:, mt * 128 : (mt + 1) * 128],
                                rhs=x_sb[kc][:, half * 512 : (half + 1) * 512],
                                start=(kc == 0),
                                stop=(kc == KT - 1),
                            )
                        for s in range(2):
                            nc.vector.tensor_copy(
                                out=dst[mt][s][:, half * 512 : (half + 1) * 512],
                                in_=ps[s * 64 : (s + 1) * 64, :],
                            )

            emit_qk_proj(0)

            # ---- vT = x^T @ wvT: [N, MID] fp16, interleaved with ones ----
            vT_sb = [
                persist.tile([128, NUM_HEADS * 33], FP16, name=f"vT{i}", tag=f"vT{i}")
                for i in range(KC)
            ]
            for kt in range(KC):
                ps = ps_small.tile([128, 512], FP32, name="v_ps", tag="ps_sm")
                for kc in range(KT):
                    nc.tensor.matmul(
                        out=ps[:, 0:MID],
                        lhsT=x_sb[kc][:, kt * 128 : (kt + 1) * 128],
                        rhs=wvT_sb[kc][:],
                        start=(kc == 0),
                        stop=(kc == KT - 1),
                    )
                dst3 = vT_sb[kt][:].rearrange("p (h c) -> p h c", h=NUM_HEADS)
                src3 = ps[:, 0:MID].rearrange("p (h c) -> p h c", h=NUM_HEADS)
                nc.vector.tensor_copy(out=dst3[:, :, 0:32], in_=src3)
                nc.vector.memset(dst3[:, :, 32:33], 1.0)

            # ---- attention ----
            attn_mid = [
                persist.tile([128, N], FP16, name=f"am{i}", tag=f"am{i}")
                for i in range(KT)
            ]

            def emit_out_proj(qc):
                # PSUM from the score pool: its slot is long free at the qc
                # boundary, so these MMs never block the in-order PE queue.
                q0 = qc * QW
                ps = ps_score.tile([128, 1024], FP32, name="op_ps", tag="st")
                OPC = (0, 256, 512)  # mt0/mt1 share bank0 sequentially
                for mt in range(KT):
                    c0 = OPC[mt]
                    for ch in range(KT):
                        nc.tensor.matmul(
                            out=ps[:, c0 : c0 + 256],
                            lhsT=wpT_sb[ch][:, mt * 128 : (mt + 1) * 128],
                            rhs=attn_mid[ch][:, q0 : q0 + QW],
                            start=(ch == 0),
                            stop=(ch == KT - 1),
                        )
                for mt in range(KT):
                    c0 = OPC[mt]
                    ob = ob_pool.tile([128, 256], FP32, name=f"ob{mt}", tag="ob")
                    nc.vector.tensor_copy(out=ob[:], in_=ps[:, c0 : c0 + 256])
                    nc.sync.dma_start(
                        out=out_d[mt * 128 : (mt + 1) * 128, q0 : q0 + QW],
                        in_=ob[:],
                    )

            pending = [[]]

            def attention_unit(qc, quad):
                q0 = qc * QW
                ebts = []
                for kcp in range(KC // 2):
                    ebt = ebt_pool.tile([128, 2048], FP16, name=f"ebt{kcp}", tag="ebt")
                    nc.sync.dma_start(out=ebt[:], in_=expB_d[quad, qc, kcp])
                    ebts.append(ebt)

                # one PSUM bank per head: single accumulation group per bank
                avs = [
                    ps_small.tile([128, 512], FP32, name=f"av{hh}", tag="ps_sm")
                    for hh in range(4)
                ]
                ats = [None] * KC
                ars = [None]

                def emit_av(kc):
                    at = ats[kc]
                    coff = (kc % 2) * 1024
                    for hh in range(4):
                        h = 4 * quad + hh
                        base = 64 * (hh % 2)
                        nc.tensor.matmul(
                            out=avs[hh][base : base + 33, 0:256],
                            lhsT=vT_sb[kc][:, h * 33 : h * 33 + 33],
                            rhs=at[:, coff + COLOF[hh] : coff + COLOF[hh] + 256],
                            start=(kc == 0),
                            stop=(kc == KC - 1),
                            tile_position=(0, base),
                        )

                for kc in range(KC):
                    kcp, kcj = kc // 2, kc % 2
                    st = ps_score.tile([128, 1024], FP32, name="st", tag="st")
                    # col layout: bank0 = [hh0 | hh2], bank1 = [hh1 | hh3];
                    # (hh0,hh1) run concurrently on distinct banks, then
                    # (hh2,hh3) reuse row groups 0/32 -> serialize after.
                    for hh in (0, 1, 2, 3):
                        s, r = hh // 2, (hh % 2) * 32
                        c0 = COLOF[hh]
                        nc.tensor.matmul(
                            out=st[:, c0 : c0 + 256],
                            lhsT=k_sb[quad][s][
                                r : r + 32, kc * 128 : (kc + 1) * 128
                            ],
                            rhs=q_sb[quad][s][r : r + 32, q0 : q0 + QW],
                            start=True,
                            stop=True,
                            tile_position=(r, 0),
                        )
                    # AV lags 3 kc so the pair-fused `at` is ready; 4-MM
                    # batches fit the PE wait queue without blocking scores.
                    if kc == 0:
                        while pending[0]:
                            pending[0].pop(0)()
                    if kc >= 3:
                        emit_av(kc - 3)
                    if kcj == 0:
                        ar = ar_pool.tile([128, 2048], FP16, name="ar", tag="ar")
                        ars[0] = ar
                    else:
                        ar = ars[0]
                    nc.scalar.activation(
                        out=ar[:, kcj * 1024 : (kcj + 1) * 1024],
                        in_=st[:],
                        func=mybir.ActivationFunctionType.Exp,
                        scale=4.0,
                    )
                    if kcj == 1:
                        at = at_pool.tile([128, 2048], FP16, name="at", tag="at")
                        nc.vector.tensor_tensor(
                            at[:], ar[:], ebts[kcp][:], mybir.AluOpType.mult
                        )
                        ats[2 * kcp] = at
                        ats[2 * kcp + 1] = at

                def norm():
                    # per-head pipelined normalize: each head's bank is
                    # released ~2us after the last AV instead of all four
                    # waiting for a batched chain.
                    for hh in range(4):
                        base = 64 * (hh % 2)
                        den = small.tile([1, 256], FP32, name=f"den{hh}", tag="den")
                        nc.vector.tensor_copy(
                            out=den[:], in_=avs[hh][base + 32 : base + 33, 0:256]
                        )
                        dsb = small.tile([1, 256], FP32, name=f"dsb{hh}", tag="dsb")
                        nc.vector.reciprocal_approx_fast(out=dsb[:], in_=den[:])
                        rb = small.tile([32, 256], FP32, name=f"rb{hh}", tag="rb")
                        nc.gpsimd.partition_broadcast(rb[:], dsb[0:1, :], channels=32)
                        nc.vector.tensor_tensor(
                            attn_mid[quad][hh * 32 : (hh + 1) * 32, q0 : q0 + QW],
                            avs[hh][base : base + 32, 0:256],
                            rb[:],
                            mybir.AluOpType.mult,
                        )

                pending[0] = [
                    lambda: emit_av(KC - 3),
                    lambda: emit_av(KC - 2),
                    lambda: emit_av(KC - 1),
                    norm,
                ]

            for qc in range(QC):
                for quad in range(NQUAD):
                    attention_unit(qc, quad)
                    # overlap the remaining projections with early attention
                    if qc == 0 and quad < 2:
                        emit_qk_proj(quad + 1)
                # out-proj one qc behind (deps long resolved -> no PE stall)
                if qc > 0:
                    emit_out_proj(qc - 1)
            for fn in pending[0]:
                fn()
            pending[0] = []
            emit_out_proj(QC - 1)
    nc.compile()
    return nc


def _prep_host(x, wq, bq, wkv, bkv, wproj, bproj, bias_table, rel_index):
    """Host-side input prep shared by all cores (weights / bias tables)."""
    wq = np.asarray(wq, np.float32) * np.float32(SCALE / 4.0)
    wkv = np.asarray(wkv, np.float32)
    wqT = np.ascontiguousarray(wq.T).astype(np.float16)
    wkT = np.ascontiguousarray(wkv[:MID].T).astype(np.float16)
    wvT = np.ascontiguousarray(wkv[MID:].T).astype(np.float16)
    wpT = np.ascontiguousarray(np.asarray(wproj, np.float32).T).astype(np.float16)
    # rel bias -> exp(bias) (exp-trick): expBT[h, j, i] = exp(B[i, j, h])
    bt = np.asarray(bias_table, np.float32)
    ri = np.asarray(rel_index, np.int64)
    Bfull = bt[ri.reshape(-1)].reshape(N, N, NUM_HEADS)  # i, j, h
    expBT = np.exp(Bfull.transpose(2, 1, 0)).astype(np.float16)  # h, j, i
    # -> [quad][qc][kcp][key][kcj*1024 + hh*256 + q], [128,2048] contiguous
    expBTr = np.ascontiguousarray(
        expBT.reshape(NQUAD, 4, KC // 2, 2, 128, QC, QW)[:, (0, 2, 1, 3)].transpose(
            0, 5, 2, 4, 3, 1, 6
        )
    ).reshape(NQUAD, QC, KC // 2, 128, 2048)
    return wqT, wkT, wvT, wpT, expBTr


def _install_ntff_hook():
    """The image's antenv lacks axon_hooks; reconstruct it so trace=True works."""
    import types, importlib.util

    try:
        from antenv.axon_hooks import get_axon_ntff_profile_hook  # noqa

        return
    except ImportError:
        pass
    import antenv

    mod = types.ModuleType("antenv.axon_hooks")
    _state = {"hook": None}
    mod.set_axon_ntff_profile_hook = lambda h: _state.__setitem__("hook", h)
    mod.get_axon_ntff_profile_hook = lambda: _state["hook"]
    sys.modules["antenv.axon_hooks"] = mod
    antenv.axon_hooks = mod

    spec = importlib.util.spec_from_file_location(
        "trn_boot", "/root/.axon_site/trn_agent_boot/trn_boot.py"
    )
    tb = importlib.util.module_from_spec(spec)
    spec.loader.exec_module(tb)
    mod.set_axon_ntff_profile_hook(
        tb._ntff_profile_via_ctypes("/opt/axon/libaxon_pjrt.so")
    )


def _run(inputs, trace=False):
    if trace:
        _install_ntff_hook()
    if "nc" not in _CACHE:
        _CACHE["nc"] = _emit_program()
    nc = _CACHE["nc"]

    x = np.asarray(inputs["x"], np.float32)
    wqT, wkT, wvT, wpT, expBTr = _prep_host(**inputs)

    in_maps = []
    for b in range(NCORES):
        in_maps.append(
            {
                "x": np.ascontiguousarray(x[b].reshape(DIM, N)).astype(np.float16),
                "wqT": wqT,
                "wkT": wkT,
                "wvT": wvT,
                "wpT": wpT,
                "expBTr": expBTr,
            }
        )
    res = run_bass_kernel_spmd(nc, in_maps, list(range(NCORES)), trace=trace)
    out = np.stack(
        [np.asarray(res.results[b]["out"]).reshape(DIM, 32, 32) for b in range(B)]
    )
    return out.astype(np.float32), res


def kernel(**inputs) -> np.ndarray:
    out, _ = _run(inputs, trace=False)
    return out


def kernel_traced(**inputs):
    """Returns (out, BassKernelResults) with profiling enabled."""
    return _run(inputs, trace=True)
